# revision 16
# baseline (speedup 1.0000x reference)
"""GATv2 (3-layer, DGL-style, share_weights, elu) on 8 Trainium2 NeuronCores.

Strategy (matches the sharding hint):
  - Nodes are partitioned into 8 contiguous shards (graph_ids is sorted, so
    this is a graph/node-segment partition). Each core owns the incoming
    edges of its node shard (edges bucketed by dst).
  - Weights are replicated. Per layer: each core projects its node shard
    (h @ W), an AllGather replicates the projected features to every core,
    then each core runs gather -> attention -> edge-softmax -> aggregation
    for its own dst nodes only.
  - Edge softmax skips the max-subtraction (scores are O(1); exp is safe in
    fp32 and the result is mathematically identical).
  - Per-node segmented sums are done on the tensor engine with per-chunk
    one-hot matrices built on the vector engine from precomputed
    tile-local dst indices (edges are sorted by dst on the host).
  - The only cross-core communication is one AllGather per layer; the final
    per-graph mean is combined on the host from per-core partial sums.
"""

import os
import sys

import numpy as np

sys.path.insert(0, "/opt/trn_rl_repo")

from concourse import bacc, bass, mybir, tile  # noqa: E402

f32 = mybir.dt.float32
i32 = mybir.dt.int32
AF = mybir.ActivationFunctionType
ALU = mybir.AluOpType
P = 128

# Problem constants (hardcoded per the task contract).
N_NODES = 50000
N_EDGES = 600000
HEADS = 4
DH = 64
D = HEADS * DH  # 256
IN0 = 128
NUM_GRAPHS = 8
N_CORES = 8
SCW = 4  # 128-edge chunks fused per DVE/ACT super-chunk


def _build_nc(cfg):
    """Build the SPMD Bass program (identical on every core)."""
    NPC = cfg["nodes_per_core"]
    NTOT = cfg["n_nodes_total"]
    CORES = cfg["n_cores"]
    G = cfg["n_graphs"]
    K = cfg["chunks_per_tile"]  # list, per node-tile chunk count (uniform across cores)
    lrelu_act = cfg.get("lrelu_act", True)
    T = len(K)
    offs = np.concatenate([[0], np.cumsum(K)]).astype(int)
    TC = int(offs[-1])

    nc = bacc.Bacc("TRN2", target_bir_lowering=False, debug=False)

    feat_in = nc.declare_dram_parameter("feat", [NPC, IN0], f32, isOutput=False)
    Wp = [
        nc.declare_dram_parameter(f"W{l}", [IN0 if l == 0 else D, D], f32, isOutput=False)
        for l in range(3)
    ]
    awp = [
        nc.declare_dram_parameter(f"aw{l}", [P, SCW * D], f32, isOutput=False)
        for l in range(3)
    ]
    srcp = nc.declare_dram_parameter("src_idx", [P, TC], i32, isOutput=False)
    dstp = nc.declare_dram_parameter("dst_idx", [P, TC], i32, isOutput=False)
    dstlp = nc.declare_dram_parameter("dstl", [P, TC], f32, isOutput=False)
    gohp = nc.declare_dram_parameter("goh", [P, T * G], f32, isOutput=False)
    iotap = nc.declare_dram_parameter("iota", [P, P], f32, isOutput=False)
    idenp = nc.declare_dram_parameter("iden", [P, P], f32, isOutput=False)
    outp = nc.declare_dram_parameter("gsum", [G, D], f32, isOutput=True)
    debug = cfg.get("debug", False)
    if debug:
        d_agin0 = nc.declare_dram_parameter("d_agin0", [NPC, D], f32, isOutput=True)
        d_ff0 = nc.declare_dram_parameter("d_ff0", [NTOT, D], f32, isOutput=True)
        d_h1 = nc.declare_dram_parameter("d_h1", [NPC, D], f32, isOutput=True)

    h_dram = [feat_in, None, None, None]
    for l in (1, 2, 3):
        h_dram[l] = nc.dram_tensor(f"h{l}", [NPC, D], f32)
    ag_in = [nc.dram_tensor(f"agin{l}", [NPC, D], f32) for l in range(3)]
    # Shared addr space is the fast path for HBM-HBM AllGather (supported >4 cores)
    ff_kw = {"addr_space": "Shared"} if CORES > 4 else {}
    feat_full = [nc.dram_tensor(f"ff{l}", [NTOT, D], f32, **ff_kw) for l in range(3)]
    rg = [list(range(CORES))]

    with tile.TileContext(nc) as tc:
        with (
            tc.tile_pool(name="consts", bufs=1) as cp,
            tc.tile_pool(name="edge", bufs=2) as ep,
            tc.tile_pool(name="small", bufs=3) as sp,
            tc.tile_pool(name="node", bufs=2) as npo,
            tc.tile_pool(name="ps", bufs=2, space="PSUM") as pp,
            tc.tile_pool(name="psg", bufs=1, space="PSUM") as pg,
        ):
            iota_t = cp.tile([P, P], f32, tag="iota")
            nc.sync.dma_start(iota_t[:], iotap[:])
            iden_t = cp.tile([P, P], f32, tag="iden")
            nc.sync.dma_start(iden_t[:], idenp[:])
            aw_t = []
            for l in range(3):
                t_ = cp.tile([P, SCW * D], f32, tag=f"aw{l}")
                nc.sync.dma_start(t_[:], awp[l][:])
                aw_t.append(t_)
            W_t = []
            for l in range(3):
                ind = IN0 if l == 0 else D
                chunks = []
                for k in range(ind // P):
                    wt = cp.tile([P, D], f32, tag=f"W{l}_{k}")
                    nc.sync.dma_start(wt[:], Wp[l][k * P : (k + 1) * P, :])
                    chunks.append(wt)
                W_t.append(chunks)
            goh_t = cp.tile([P, T * G], f32, tag="goh")
            nc.sync.dma_start(goh_t[:], gohp[:])
            gacc = cp.tile([G, D], f32, tag="gacc")
            nc.vector.memset(gacc[:], 0.0)
            # Pre-touch iden on the PE so later transpose instructions never
            # need a second sync wait (transpose encoding carries only one).
            # Dedicated PSUM tag, never reused -> no release dependency.
            ps_warm = pg.tile([P, P], f32, tag="warm", space="PSUM")
            nc.tensor.transpose(ps_warm[:], iden_t[:], iden_t[:])

            for l in range(3):
                ind = IN0 if l == 0 else D
                # ---- projection: feat_proj = h @ W_l for the local shard ----
                for t in range(T):
                    pt = min(P, NPC - t * P)
                    h_t = npo.tile([P, ind], f32, tag="h")
                    nc.sync.dma_start(h_t[:pt, :], h_dram[l][t * P : t * P + pt, :])
                    ps_proj = pp.tile([P, D], f32, tag="proj", space="PSUM")
                    nk = ind // P
                    for k in range(nk):
                        ps_tr = pp.tile([P, P], f32, tag="tr", space="PSUM")
                        nc.tensor.transpose(
                            ps_tr[:, :pt], h_t[:pt, k * P : (k + 1) * P], iden_t[:pt, :pt]
                        )
                        hT = npo.tile([P, P], f32, tag="hT")
                        nc.scalar.copy(hT[:, :pt], ps_tr[:, :pt])
                        nc.tensor.matmul(
                            ps_proj[:pt, :],
                            lhsT=hT[:, :pt],
                            rhs=W_t[l][k][:],
                            start=(k == 0),
                            stop=(k == nk - 1),
                        )
                    proj_t = npo.tile([P, D], f32, tag="proj_s")
                    nc.scalar.copy(proj_t[:pt, :], ps_proj[:pt, :])
                    nc.sync.dma_start(ag_in[l][t * P : t * P + pt, :], proj_t[:pt, :])
                    if debug and l == 0:
                        nc.sync.dma_start(
                            d_agin0[t * P : t * P + pt, :], proj_t[:pt, :]
                        )
                # ---- replicate projected features ----
                nc.gpsimd.collective_compute(
                    "AllGather",
                    ALU.bypass,
                    ins=[ag_in[l][:]],
                    outs=[feat_full[l][:]],
                    replica_groups=rg,
                )
                # ---- edge phase, per dst node tile ----
                for t in range(T):
                    pt = min(P, NPC - t * P)
                    kt = int(K[t])
                    off = int(offs[t])
                    out_t = npo.tile([P, D], f32, tag="out_t")
                    if kt > 0:
                        sidx = sp.tile([P, kt], i32, tag="sidx")
                        nc.sync.dma_start(sidx[:], srcp[:, off : off + kt])
                        didx = sp.tile([P, kt], i32, tag="didx")
                        nc.sync.dma_start(didx[:], dstp[:, off : off + kt])
                        dstl_t = sp.tile([P, kt], f32, tag="dstl")
                        nc.sync.dma_start(dstl_t[:], dstlp[:, off : off + kt])
                        ps_agg = pp.tile([P, D + HEADS], f32, tag="agg", space="PSUM")
                        first = True
                        c = 0
                        while c < kt:
                            m = min(SCW, kt - c)
                            W_ = m * D
                            # HW indirect DMA honors only one offset per
                            # partition -> one gather per 128-edge chunk.
                            fs = ep.tile([P, SCW * D], f32, tag="fs")
                            fd = ep.tile([P, SCW * D], f32, tag="fd")
                            for j in range(m):
                                nc.gpsimd.indirect_dma_start(
                                    out=fs[:, j * D : (j + 1) * D],
                                    out_offset=None,
                                    in_=feat_full[l][:],
                                    in_offset=bass.IndirectOffsetOnAxis(
                                        ap=sidx[:, c + j : c + j + 1], axis=0
                                    ),
                                )
                                nc.gpsimd.indirect_dma_start(
                                    out=fd[:, j * D : (j + 1) * D],
                                    out_offset=None,
                                    in_=feat_full[l][:],
                                    in_offset=bass.IndirectOffsetOnAxis(
                                        ap=didx[:, c + j : c + j + 1], axis=0
                                    ),
                                )
                            xa = ep.tile([P, SCW * D], f32, tag="xa")
                            nc.vector.tensor_tensor(
                                out=xa[:, :W_], in0=fs[:, :W_], in1=fd[:, :W_], op=ALU.add
                            )
                            lr = ep.tile([P, SCW * D], f32, tag="lr")
                            if lrelu_act:
                                nc.scalar.activation(
                                    lr[:, :W_], xa[:, :W_], AF.Lrelu, alpha=0.2
                                )
                            else:
                                nc.scalar.mul(lr[:, :W_], xa[:, :W_], 0.2)
                                nc.vector.tensor_tensor(
                                    out=lr[:, :W_], in0=lr[:, :W_], in1=xa[:, :W_],
                                    op=ALU.max,
                                )
                            mm_ = ep.tile([P, SCW * D], f32, tag="mm")
                            nc.vector.tensor_tensor(
                                out=mm_[:, :W_], in0=lr[:, :W_], in1=aw_t[l][:, :W_],
                                op=ALU.mult,
                            )
                            scores = sp.tile([P, SCW * HEADS], f32, tag="sc")
                            nc.vector.reduce_sum(
                                out=scores[:, : m * HEADS],
                                in_=mm_[:, :W_].rearrange("p (g d) -> p g d", d=DH),
                                axis=mybir.AxisListType.X,
                            )
                            ex = sp.tile([P, SCW * HEADS], f32, tag="ex")
                            nc.scalar.activation(
                                ex[:, : m * HEADS], scores[:, : m * HEADS], AF.Exp
                            )
                            DE = D + HEADS
                            msg = ep.tile([P, SCW * DE], f32, tag="msg")
                            msg3 = msg[:, : m * DE].rearrange("p (g r) -> p g r", r=DE)
                            nc.vector.tensor_tensor(
                                out=msg3[:, :, :D].rearrange(
                                    "p g (h d) -> p g h d", d=DH
                                ),
                                in0=fs[:, :W_].rearrange(
                                    "p (g h d) -> p g h d", h=HEADS, d=DH
                                ),
                                in1=ex[:, : m * HEADS]
                                .rearrange("p (g h) -> p g h", h=HEADS)
                                .to_broadcast([P, m, HEADS, DH]),
                                op=ALU.mult,
                            )
                            nc.vector.tensor_copy(
                                msg3[:, :, D:DE],
                                ex[:, : m * HEADS].rearrange("p (g h) -> p g h", h=HEADS),
                            )
                            for j in range(m):
                                oh = sp.tile([P, P], f32, tag="oh")
                                nc.vector.tensor_tensor(
                                    out=oh[:],
                                    in0=dstl_t[:, c + j : c + j + 1].to_broadcast([P, P]),
                                    in1=iota_t[:],
                                    op=ALU.is_equal,
                                )
                                nc.tensor.matmul(
                                    ps_agg[:, :DE],
                                    lhsT=oh[:],
                                    rhs=msg[:, j * DE : (j + 1) * DE],
                                    start=first,
                                    stop=(c + j == kt - 1),
                                )
                                first = False
                            c += m
                        denom = sp.tile([P, HEADS], f32, tag="den")
                        nc.vector.tensor_scalar_max(
                            denom[:pt, :], ps_agg[:pt, D : D + HEADS], 1e-30
                        )
                        recip = sp.tile([P, HEADS], f32, tag="rcp")
                        nc.vector.reciprocal(recip[:pt, :], denom[:pt, :])
                        for h in range(HEADS):
                            nc.vector.tensor_scalar_mul(
                                out_t[:pt, h * DH : (h + 1) * DH],
                                ps_agg[:pt, h * DH : (h + 1) * DH],
                                recip[:pt, h : h + 1],
                            )
                    else:
                        nc.vector.memset(out_t[:pt, :], 0.0)
                    if l > 0:
                        hres = npo.tile([P, D], f32, tag="hres")
                        nc.sync.dma_start(hres[:pt, :], h_dram[l][t * P : t * P + pt, :])
                        nc.vector.tensor_tensor(
                            out=out_t[:pt, :], in0=out_t[:pt, :], in1=hres[:pt, :],
                            op=ALU.add,
                        )
                    # elu(x) = expm1(min(x,0)) + max(x,0)
                    mneg = npo.tile([P, D], f32, tag="mneg")
                    nc.vector.tensor_scalar_min(mneg[:pt, :], out_t[:pt, :], 0.0)
                    epos = npo.tile([P, D], f32, tag="epos")
                    nc.vector.tensor_scalar(
                        out=epos[:pt, :], in0=out_t[:pt, :], scalar1=0.0, scalar2=-1.0,
                        op0=ALU.max, op1=ALU.add,
                    )
                    eneg = npo.tile([P, D], f32, tag="eneg")
                    nc.scalar.activation(eneg[:pt, :], mneg[:pt, :], AF.Exp)
                    hn = npo.tile([P, D], f32, tag="hn")
                    nc.vector.tensor_tensor(
                        out=hn[:pt, :], in0=eneg[:pt, :], in1=epos[:pt, :], op=ALU.add
                    )
                    nc.sync.dma_start(h_dram[l + 1][t * P : t * P + pt, :], hn[:pt, :])
                    if l == 2:
                        ps_g = pg.tile([G, D], f32, tag="gps", space="PSUM")
                        nc.tensor.matmul(
                            ps_g[:, :],
                            lhsT=goh_t[:pt, t * G : (t + 1) * G],
                            rhs=hn[:pt, :],
                            start=True, stop=True,
                        )
                        nc.vector.tensor_tensor(
                            out=gacc[:], in0=gacc[:], in1=ps_g[:], op=ALU.add
                        )
            nc.sync.dma_start(outp[:], gacc[:])
            if debug:
                nc.sync.dma_start(d_ff0[:], feat_full[0][:])
                nc.sync.dma_start(d_h1[:], h_dram[1][:])
    nc.compile()
    return nc


def _preprocess(src, dst, graph_ids, n_nodes, n_cores, n_graphs):
    """Host-side: bucket dst-sorted edges into per-core node tiles and
    128-edge chunks; build index / one-hot-support arrays."""
    npc = n_nodes // n_cores
    tiles = (npc + P - 1) // P
    order = np.argsort(dst, kind="stable")
    src_s = src[order].astype(np.int32)
    dst_s = dst[order].astype(np.int32)

    # edge range for every (core, tile): node boundaries every 128 nodes
    bounds = np.searchsorted(
        dst_s, np.arange(0, n_cores * tiles + 1) * 0
    )  # placeholder, replaced below
    node_bounds = []
    for c in range(n_cores):
        for t in range(tiles):
            node_bounds.append(c * npc + t * P)
    node_bounds.append(n_nodes)
    bounds = np.searchsorted(dst_s, np.asarray(node_bounds))

    cnt = (bounds[1:] - bounds[:-1]).reshape(n_cores, tiles)
    K = np.maximum(1, (cnt + P - 1) // P).max(axis=0)  # per-tile chunks, core-uniform
    offs = np.concatenate([[0], np.cumsum(K)]).astype(int)
    tc_total = int(offs[-1])

    src_idx = np.zeros((n_cores, P, tc_total), np.int32)
    dst_idx = np.zeros((n_cores, P, tc_total), np.int32)
    dstl = np.full((n_cores, P, tc_total), -1.0, np.float32)
    for c in range(n_cores):
        for t in range(tiles):
            e0 = bounds[c * tiles + t]
            e1 = bounds[c * tiles + t + 1]
            n = e1 - e0
            if n == 0:
                continue
            j = np.arange(n)
            lane = j % P
            col = offs[t] + j // P
            src_idx[c, lane, col] = src_s[e0:e1]
            dst_idx[c, lane, col] = dst_s[e0:e1]
            dstl[c, lane, col] = (dst_s[e0:e1] - (c * npc + t * P)).astype(np.float32)

    # graph one-hot per node tile
    goh = np.zeros((n_cores, P, tiles * n_graphs), np.float32)
    for c in range(n_cores):
        for t in range(tiles):
            lo = c * npc + t * P
            hi = min(lo + P, (c + 1) * npc)
            ids = graph_ids[lo:hi]
            goh[c, np.arange(hi - lo), t * n_graphs + ids] = 1.0

    return {
        "K": [int(x) for x in K],
        "offs": offs,
        "src_idx": src_idx,
        "dst_idx": dst_idx,
        "dstl": dstl,
        "goh": goh,
        "npc": npc,
        "tiles": tiles,
    }


def _make_in_maps(inputs, pre, n_cores, n_graphs):
    feat = np.ascontiguousarray(inputs["feat"], dtype=np.float32)
    npc = pre["npc"]
    iota = np.tile(np.arange(P, dtype=np.float32), (P, 1))
    iden = np.eye(P, dtype=np.float32)
    aws = [
        np.tile(np.asarray(inputs[f"a{l}"], np.float32).reshape(1, D), (P, SCW))
        for l in range(3)
    ]
    in_maps = []
    for c in range(n_cores):
        in_maps.append(
            {
                "feat": feat[c * npc : (c + 1) * npc],
                "W0": np.ascontiguousarray(inputs["W0"], np.float32),
                "W1": np.ascontiguousarray(inputs["W1"], np.float32),
                "W2": np.ascontiguousarray(inputs["W2"], np.float32),
                "aw0": aws[0],
                "aw1": aws[1],
                "aw2": aws[2],
                "src_idx": np.ascontiguousarray(pre["src_idx"][c]),
                "dst_idx": np.ascontiguousarray(pre["dst_idx"][c]),
                "dstl": np.ascontiguousarray(pre["dstl"][c]),
                "goh": np.ascontiguousarray(pre["goh"][c]),
                "iota": iota,
                "iden": iden,
            }
        )
    return in_maps


def kernel(**inputs):
    from concourse.bass_utils import run_bass_kernel_spmd

    src = np.asarray(inputs["src"], np.int32)
    dst = np.asarray(inputs["dst"], np.int32)
    graph_ids = np.asarray(inputs["graph_ids"], np.int32)

    pre = _preprocess(src, dst, graph_ids, N_NODES, N_CORES, NUM_GRAPHS)
    cfg = {
        "nodes_per_core": pre["npc"],
        "n_nodes_total": N_NODES,
        "n_cores": N_CORES,
        "n_graphs": NUM_GRAPHS,
        "chunks_per_tile": pre["K"],
        "lrelu_act": False,
    }
    nc = _build_nc(cfg)
    in_maps = _make_in_maps(inputs, pre, N_CORES, NUM_GRAPHS)
    res = run_bass_kernel_spmd(nc, in_maps, list(range(N_CORES)))
    total = np.zeros((NUM_GRAPHS, D), np.float32)
    for r in res.results:
        total += r["gsum"]
    counts = np.bincount(graph_ids, minlength=NUM_GRAPHS).astype(np.float32)
    return (total / np.maximum(counts, 1.0)[:, None]).astype(np.float32)


# revision 29
# speedup vs baseline: 12.0114x; 12.0114x over previous
"""GATv2 (3-layer, DGL-style, share_weights, elu) on 8 Trainium2 NeuronCores.

Strategy (matches the sharding hint):
  - Nodes are partitioned into 8 contiguous shards (graph_ids is sorted, so
    this is a graph/node-segment partition). Each core owns the incoming
    edges of its node shard (edges bucketed by dst).
  - Weights are replicated. Per layer: each core projects its node shard
    (h @ W), an AllGather replicates the projected features to every core,
    then each core runs gather -> attention -> edge-softmax -> aggregation
    for its own dst nodes only.
  - Edge softmax skips the max-subtraction (scores are O(1); exp is safe in
    fp32 and the result is mathematically identical).
  - Per-node segmented sums are done on the tensor engine with per-chunk
    one-hot matrices built on the vector engine from precomputed
    tile-local dst indices (edges are sorted by dst on the host).
  - The only cross-core communication is one AllGather per layer; the final
    per-graph mean is combined on the host from per-core partial sums.
"""

import os
import sys

import numpy as np

sys.path.insert(0, "/opt/trn_rl_repo")

from concourse import bacc, bass, mybir, tile  # noqa: E402

f32 = mybir.dt.float32
i32 = mybir.dt.int32
AF = mybir.ActivationFunctionType
ALU = mybir.AluOpType
P = 128

# Problem constants (hardcoded per the task contract).
N_NODES = 50000
N_EDGES = 600000
HEADS = 4
DH = 64
D = HEADS * DH  # 256
IN0 = 128
NUM_GRAPHS = 8
N_CORES = 8
SCW = 4  # 128-edge chunks fused per DVE/ACT super-chunk


def _build_nc(cfg):
    """Build the SPMD Bass program (identical on every core)."""
    NPC = cfg["nodes_per_core"]
    NTOT = cfg["n_nodes_total"]
    CORES = cfg["n_cores"]
    G = cfg["n_graphs"]
    K = cfg["chunks_per_tile"]  # list, per node-tile chunk count (uniform across cores)
    lrelu_act = cfg.get("lrelu_act", True)
    T = len(K)
    offs = np.concatenate([[0], np.cumsum(K)]).astype(int)
    TC = int(offs[-1])

    nc = bacc.Bacc("TRN2", target_bir_lowering=False, debug=False)

    feat_in = nc.declare_dram_parameter("feat", [NPC, IN0], f32, isOutput=False)
    Wp = [
        nc.declare_dram_parameter(f"W{l}", [IN0 if l == 0 else D, D], f32, isOutput=False)
        for l in range(3)
    ]
    awp = [
        nc.declare_dram_parameter(f"aw{l}", [P, SCW * D], f32, isOutput=False)
        for l in range(3)
    ]
    srcp = nc.declare_dram_parameter("src_idx", [P, TC], i32, isOutput=False)
    dstp = nc.declare_dram_parameter("dst_idx", [P, TC], i32, isOutput=False)
    dstlp = nc.declare_dram_parameter("dstl", [P, TC], f32, isOutput=False)
    gohp = nc.declare_dram_parameter("goh", [P, T * G], f32, isOutput=False)
    iotap = nc.declare_dram_parameter("iota", [P, P], f32, isOutput=False)
    idenp = nc.declare_dram_parameter("iden", [P, P], f32, isOutput=False)
    outp = nc.declare_dram_parameter("gsum", [G, D], f32, isOutput=True)
    debug = cfg.get("debug", False)
    if debug:
        d_agin0 = nc.declare_dram_parameter("d_agin0", [NPC, D], f32, isOutput=True)
        d_ff0 = nc.declare_dram_parameter("d_ff0", [NTOT, D], f32, isOutput=True)
        d_h1 = nc.declare_dram_parameter("d_h1", [NPC, D], f32, isOutput=True)

    h_dram = [feat_in, None, None, None]
    for l in (1, 2, 3):
        h_dram[l] = nc.dram_tensor(f"h{l}", [NPC, D], f32)
    ag_in = [nc.dram_tensor(f"agin{l}", [NPC, D], f32) for l in range(3)]
    # Shared addr space is the fast path for HBM-HBM AllGather (supported >4 cores)
    ff_kw = {"addr_space": "Shared"} if CORES > 4 else {}
    feat_full = [nc.dram_tensor(f"ff{l}", [NTOT, D], f32, **ff_kw) for l in range(3)]
    rg = [list(range(CORES))]

    with tile.TileContext(nc) as tc:
        with (
            tc.tile_pool(name="consts", bufs=1) as cp,
            tc.tile_pool(name="edge", bufs=2) as ep,
            tc.tile_pool(name="small", bufs=3) as sp,
            tc.tile_pool(name="node", bufs=2) as npo,
            tc.tile_pool(name="ps", bufs=2, space="PSUM") as pp,
            tc.tile_pool(name="psg", bufs=1, space="PSUM") as pg,
        ):
            iota_t = cp.tile([P, P], f32, tag="iota")
            nc.sync.dma_start(iota_t[:], iotap[:])
            iden_t = cp.tile([P, P], f32, tag="iden")
            nc.sync.dma_start(iden_t[:], idenp[:])
            aw_t = []
            for l in range(3):
                t_ = cp.tile([P, SCW * D], f32, tag=f"aw{l}")
                nc.sync.dma_start(t_[:], awp[l][:])
                aw_t.append(t_)
            W_t = []
            for l in range(3):
                ind = IN0 if l == 0 else D
                chunks = []
                for k in range(ind // P):
                    wt = cp.tile([P, D], f32, tag=f"W{l}_{k}")
                    nc.sync.dma_start(wt[:], Wp[l][k * P : (k + 1) * P, :])
                    chunks.append(wt)
                W_t.append(chunks)
            goh_t = cp.tile([P, T * G], f32, tag="goh")
            nc.sync.dma_start(goh_t[:], gohp[:])
            gacc = cp.tile([G, D], f32, tag="gacc")
            nc.vector.memset(gacc[:], 0.0)

            for l in range(3):
                ind = IN0 if l == 0 else D
                # ---- projection: feat_proj = h @ W_l for the local shard ----
                for t in range(T):
                    pt = min(P, NPC - t * P)
                    h_t = npo.tile([P, ind], f32, tag="h")
                    nc.sync.dma_start(h_t[:pt, :], h_dram[l][t * P : t * P + pt, :])
                    ps_proj = pp.tile(
                        [P, D], f32, tag="proj", space="PSUM",
                        bufs=1 if cfg.get("dst_expand") else 2,
                    )
                    nk = ind // P
                    for k in range(nk):
                        ps_tr = pp.tile([P, P], f32, tag="tr", space="PSUM")
                        nc.tensor.transpose(
                            ps_tr[:, :pt], h_t[:pt, k * P : (k + 1) * P], iden_t[:pt, :pt]
                        )
                        hT = npo.tile([P, P], f32, tag="hT")
                        nc.scalar.copy(hT[:, :pt], ps_tr[:, :pt])
                        nc.tensor.matmul(
                            ps_proj[:pt, :],
                            lhsT=hT[:, :pt],
                            rhs=W_t[l][k][:],
                            start=(k == 0),
                            stop=(k == nk - 1),
                        )
                    proj_t = npo.tile([P, D], f32, tag="proj_s")
                    nc.scalar.copy(proj_t[:pt, :], ps_proj[:pt, :])
                    nc.sync.dma_start(ag_in[l][t * P : t * P + pt, :], proj_t[:pt, :])
                    if debug and l == 0:
                        nc.sync.dma_start(
                            d_agin0[t * P : t * P + pt, :], proj_t[:pt, :]
                        )
                # ---- replicate projected features ----
                nc.gpsimd.collective_compute(
                    "AllGather",
                    ALU.bypass,
                    ins=[ag_in[l][:]],
                    outs=[feat_full[l][:]],
                    replica_groups=rg,
                )
                # ---- edge phase, per dst node tile ----
                dst_expand = cfg.get("dst_expand", False)
                for t in range(T):
                    pt = min(P, NPC - t * P)
                    kt = int(K[t])
                    off = int(offs[t])
                    out_t = npo.tile([P, D], f32, tag="out_t")
                    if cfg.get("ablate_edge"):
                        nc.vector.memset(out_t[:pt, :], 0.0)
                    elif kt > 0:
                        sidx = sp.tile([P, kt], i32, tag="sidx")
                        nc.sync.dma_start(sidx[:], srcp[:, off : off + kt])
                        if dst_expand:
                            # dst rows of this tile are contiguous: load once,
                            # expand per chunk on the PE via the one-hot.
                            nb = npo.tile([P, D], f32, tag="nb")
                            if pt < P:
                                nc.vector.memset(nb[:, :], 0.0)
                            nc.sync.dma_start(
                                nb[:pt, :], ag_in[l][t * P : t * P + pt, :]
                            )
                        else:
                            didx = sp.tile([P, kt], i32, tag="didx")
                            nc.sync.dma_start(didx[:], dstp[:, off : off + kt])
                        dstl_t = sp.tile([P, kt], f32, tag="dstl")
                        nc.sync.dma_start(dstl_t[:], dstlp[:, off : off + kt])
                        ps_agg = pp.tile([P, D + HEADS], f32, tag="agg", space="PSUM")
                        first = True
                        c = 0
                        while c < kt:
                            m = min(SCW, kt - c)
                            W_ = m * D
                            # HW indirect DMA honors only one offset per
                            # partition -> one gather per 128-edge chunk.
                            fs = ep.tile([P, SCW * D], f32, tag="fs")
                            fd = ep.tile([P, SCW * D], f32, tag="fd")
                            for j in range(m):
                                if cfg.get("ablate_src"):
                                    nc.sync.dma_start(
                                        fs[:, j * D : (j + 1) * D],
                                        feat_full[l][t * P : t * P + P, :],
                                    )
                                else:
                                    nc.gpsimd.indirect_dma_start(
                                        out=fs[:, j * D : (j + 1) * D],
                                        out_offset=None,
                                        in_=feat_full[l][:],
                                        in_offset=bass.IndirectOffsetOnAxis(
                                            ap=sidx[:, c + j : c + j + 1], axis=0
                                        ),
                                    )
                                if not cfg.get("ablate_dst") and not dst_expand:
                                    nc.gpsimd.indirect_dma_start(
                                        out=fd[:, j * D : (j + 1) * D],
                                        out_offset=None,
                                        in_=feat_full[l][:],
                                        in_offset=bass.IndirectOffsetOnAxis(
                                            ap=didx[:, c + j : c + j + 1], axis=0
                                        ),
                                    )
                            if cfg.get("ablate_dve"):
                                for j in range(m):
                                    oh = sp.tile([P, P], f32, tag="oh")
                                    nc.vector.tensor_tensor(
                                        out=oh[:],
                                        in0=dstl_t[:, c + j : c + j + 1].to_broadcast(
                                            [P, P]
                                        ),
                                        in1=iota_t[:],
                                        op=ALU.is_equal,
                                    )
                                    nc.tensor.matmul(
                                        ps_agg[:, :D],
                                        lhsT=oh[:],
                                        rhs=fs[:, j * D : (j + 1) * D],
                                        start=first,
                                        stop=(c + j == kt - 1),
                                    )
                                    nc.tensor.matmul(
                                        ps_agg[:, D : D + HEADS],
                                        lhsT=oh[:],
                                        rhs=fs[:, j * D : j * D + HEADS],
                                        start=first,
                                        stop=(c + j == kt - 1),
                                    )
                                    first = False
                                c += m
                                continue
                            oh_list = []
                            lr = ep.tile([P, SCW * D], f32, tag="lr")
                            if dst_expand:
                                # x_j = onehot_j^T @ nb + I @ fs_j  (PE builds
                                # feat[dst]+feat[src] directly in PSUM)
                                for j in range(m):
                                    oh = sp.tile([P, P], f32, tag="oh", bufs=6)
                                    nc.vector.tensor_tensor(
                                        out=oh[:],
                                        in0=dstl_t[:, c + j : c + j + 1].to_broadcast(
                                            [P, P]
                                        ),
                                        in1=iota_t[:],
                                        op=ALU.is_equal,
                                    )
                                    oh_list.append(oh)
                                    ps_trE = pp.tile(
                                        [P, P], f32, tag="tr", space="PSUM"
                                    )
                                    nc.tensor.transpose(ps_trE[:], oh[:], iden_t[:])
                                    ohT = sp.tile([P, P], f32, tag="ohT")
                                    nc.scalar.copy(ohT[:], ps_trE[:])
                                    ps_x = pp.tile([P, D], f32, tag="x", space="PSUM")
                                    nc.tensor.matmul(
                                        ps_x[:], lhsT=ohT[:], rhs=nb[:],
                                        start=True, stop=False,
                                    )
                                    nc.tensor.matmul(
                                        ps_x[:], lhsT=iden_t[:],
                                        rhs=fs[:, j * D : (j + 1) * D],
                                        start=False, stop=True,
                                    )
                                    t02 = npo.tile([P, D], f32, tag="t02")
                                    nc.scalar.mul(t02[:], ps_x[:], 0.2)
                                    nc.vector.tensor_tensor(
                                        out=lr[:, j * D : (j + 1) * D],
                                        in0=t02[:], in1=ps_x[:], op=ALU.max,
                                    )
                            else:
                                xa = ep.tile([P, SCW * D], f32, tag="xa")
                                nc.vector.tensor_tensor(
                                    out=xa[:, :W_], in0=fs[:, :W_],
                                    in1=fs[:, :W_] if cfg.get("ablate_dst")
                                    else fd[:, :W_],
                                    op=ALU.add,
                                )
                                nc.scalar.mul(lr[:, :W_], xa[:, :W_], 0.2)
                                nc.vector.tensor_tensor(
                                    out=lr[:, :W_], in0=lr[:, :W_], in1=xa[:, :W_],
                                    op=ALU.max,
                                )
                            mm_ = ep.tile([P, SCW * D], f32, tag="mm")
                            nc.vector.tensor_tensor(
                                out=mm_[:, :W_], in0=lr[:, :W_], in1=aw_t[l][:, :W_],
                                op=ALU.mult,
                            )
                            scores = sp.tile([P, SCW * HEADS], f32, tag="sc")
                            nc.vector.reduce_sum(
                                out=scores[:, : m * HEADS],
                                in_=mm_[:, :W_].rearrange("p (g d) -> p g d", d=DH),
                                axis=mybir.AxisListType.X,
                            )
                            ex = sp.tile([P, SCW * HEADS], f32, tag="ex")
                            nc.scalar.activation(
                                ex[:, : m * HEADS], scores[:, : m * HEADS], AF.Exp
                            )
                            DE = D + HEADS
                            msg = ep.tile([P, SCW * DE], f32, tag="msg")
                            msg3 = msg[:, : m * DE].rearrange("p (g r) -> p g r", r=DE)
                            nc.vector.tensor_tensor(
                                out=msg3[:, :, :D].rearrange(
                                    "p g (h d) -> p g h d", d=DH
                                ),
                                in0=fs[:, :W_].rearrange(
                                    "p (g h d) -> p g h d", h=HEADS, d=DH
                                ),
                                in1=ex[:, : m * HEADS]
                                .rearrange("p (g h) -> p g h", h=HEADS)
                                .to_broadcast([P, m, HEADS, DH]),
                                op=ALU.mult,
                            )
                            nc.vector.tensor_copy(
                                msg3[:, :, D:DE],
                                ex[:, : m * HEADS].rearrange("p (g h) -> p g h", h=HEADS),
                            )
                            for j in range(m):
                                if dst_expand:
                                    oh = oh_list[j]
                                else:
                                    oh = sp.tile([P, P], f32, tag="oh")
                                    nc.vector.tensor_tensor(
                                        out=oh[:],
                                        in0=dstl_t[:, c + j : c + j + 1].to_broadcast(
                                            [P, P]
                                        ),
                                        in1=iota_t[:],
                                        op=ALU.is_equal,
                                    )
                                nc.tensor.matmul(
                                    ps_agg[:, :DE],
                                    lhsT=oh[:],
                                    rhs=msg[:, j * DE : (j + 1) * DE],
                                    start=first,
                                    stop=(c + j == kt - 1),
                                )
                                first = False
                            c += m
                        denom = sp.tile([P, HEADS], f32, tag="den")
                        nc.vector.tensor_scalar_max(
                            denom[:pt, :], ps_agg[:pt, D : D + HEADS], 1e-30
                        )
                        recip = sp.tile([P, HEADS], f32, tag="rcp")
                        nc.vector.reciprocal(recip[:pt, :], denom[:pt, :])
                        for h in range(HEADS):
                            nc.vector.tensor_scalar_mul(
                                out_t[:pt, h * DH : (h + 1) * DH],
                                ps_agg[:pt, h * DH : (h + 1) * DH],
                                recip[:pt, h : h + 1],
                            )
                    else:
                        nc.vector.memset(out_t[:pt, :], 0.0)
                    if l > 0:
                        hres = npo.tile([P, D], f32, tag="hres")
                        nc.sync.dma_start(hres[:pt, :], h_dram[l][t * P : t * P + pt, :])
                        nc.vector.tensor_tensor(
                            out=out_t[:pt, :], in0=out_t[:pt, :], in1=hres[:pt, :],
                            op=ALU.add,
                        )
                    # elu(x) = expm1(min(x,0)) + max(x,0)
                    mneg = npo.tile([P, D], f32, tag="mneg")
                    nc.vector.tensor_scalar_min(mneg[:pt, :], out_t[:pt, :], 0.0)
                    epos = npo.tile([P, D], f32, tag="epos")
                    nc.vector.tensor_scalar(
                        out=epos[:pt, :], in0=out_t[:pt, :], scalar1=0.0, scalar2=-1.0,
                        op0=ALU.max, op1=ALU.add,
                    )
                    eneg = npo.tile([P, D], f32, tag="eneg")
                    nc.scalar.activation(eneg[:pt, :], mneg[:pt, :], AF.Exp)
                    hn = npo.tile([P, D], f32, tag="hn")
                    nc.vector.tensor_tensor(
                        out=hn[:pt, :], in0=eneg[:pt, :], in1=epos[:pt, :], op=ALU.add
                    )
                    nc.sync.dma_start(h_dram[l + 1][t * P : t * P + pt, :], hn[:pt, :])
                    if l == 2:
                        ps_g = pg.tile([G, D], f32, tag="gps", space="PSUM")
                        nc.tensor.matmul(
                            ps_g[:, :],
                            lhsT=goh_t[:pt, t * G : (t + 1) * G],
                            rhs=hn[:pt, :],
                            start=True, stop=True,
                        )
                        nc.vector.tensor_tensor(
                            out=gacc[:], in0=gacc[:], in1=ps_g[:], op=ALU.add
                        )
            nc.sync.dma_start(outp[:], gacc[:])
            if debug:
                nc.sync.dma_start(d_ff0[:], feat_full[0][:])
                nc.sync.dma_start(d_h1[:], h_dram[1][:])
    nc.compile()
    return nc


def _preprocess(src, dst, graph_ids, n_nodes, n_cores, n_graphs):
    """Host-side: bucket dst-sorted edges into per-core node tiles and
    128-edge chunks; build index / one-hot-support arrays."""
    npc = n_nodes // n_cores
    tiles = (npc + P - 1) // P
    order = np.argsort(dst, kind="stable")
    src_s = src[order].astype(np.int32)
    dst_s = dst[order].astype(np.int32)

    # edge range for every (core, tile): node boundaries every 128 nodes
    bounds = np.searchsorted(
        dst_s, np.arange(0, n_cores * tiles + 1) * 0
    )  # placeholder, replaced below
    node_bounds = []
    for c in range(n_cores):
        for t in range(tiles):
            node_bounds.append(c * npc + t * P)
    node_bounds.append(n_nodes)
    bounds = np.searchsorted(dst_s, np.asarray(node_bounds))

    cnt = (bounds[1:] - bounds[:-1]).reshape(n_cores, tiles)
    K = np.maximum(1, (cnt + P - 1) // P).max(axis=0)  # per-tile chunks, core-uniform
    offs = np.concatenate([[0], np.cumsum(K)]).astype(int)
    tc_total = int(offs[-1])

    src_idx = np.zeros((n_cores, P, tc_total), np.int32)
    dst_idx = np.zeros((n_cores, P, tc_total), np.int32)
    dstl = np.full((n_cores, P, tc_total), -1.0, np.float32)
    for c in range(n_cores):
        for t in range(tiles):
            e0 = bounds[c * tiles + t]
            e1 = bounds[c * tiles + t + 1]
            n = e1 - e0
            if n == 0:
                continue
            j = np.arange(n)
            lane = j % P
            col = offs[t] + j // P
            src_idx[c, lane, col] = src_s[e0:e1]
            dst_idx[c, lane, col] = dst_s[e0:e1]
            dstl[c, lane, col] = (dst_s[e0:e1] - (c * npc + t * P)).astype(np.float32)

    # graph one-hot per node tile
    goh = np.zeros((n_cores, P, tiles * n_graphs), np.float32)
    for c in range(n_cores):
        for t in range(tiles):
            lo = c * npc + t * P
            hi = min(lo + P, (c + 1) * npc)
            ids = graph_ids[lo:hi]
            goh[c, np.arange(hi - lo), t * n_graphs + ids] = 1.0

    return {
        "K": [int(x) for x in K],
        "offs": offs,
        "src_idx": src_idx,
        "dst_idx": dst_idx,
        "dstl": dstl,
        "goh": goh,
        "npc": npc,
        "tiles": tiles,
    }


def _make_in_maps(inputs, pre, n_cores, n_graphs):
    feat = np.ascontiguousarray(inputs["feat"], dtype=np.float32)
    npc = pre["npc"]
    iota = np.tile(np.arange(P, dtype=np.float32), (P, 1))
    iden = np.eye(P, dtype=np.float32)
    aws = [
        np.tile(np.asarray(inputs[f"a{l}"], np.float32).reshape(1, D), (P, SCW))
        for l in range(3)
    ]
    in_maps = []
    for c in range(n_cores):
        in_maps.append(
            {
                "feat": feat[c * npc : (c + 1) * npc],
                "W0": np.ascontiguousarray(inputs["W0"], np.float32),
                "W1": np.ascontiguousarray(inputs["W1"], np.float32),
                "W2": np.ascontiguousarray(inputs["W2"], np.float32),
                "aw0": aws[0],
                "aw1": aws[1],
                "aw2": aws[2],
                "src_idx": np.ascontiguousarray(pre["src_idx"][c]),
                "dst_idx": np.ascontiguousarray(pre["dst_idx"][c]),
                "dstl": np.ascontiguousarray(pre["dstl"][c]),
                "goh": np.ascontiguousarray(pre["goh"][c]),
                "iota": iota,
                "iden": iden,
            }
        )
    return in_maps


def _full_cfg(pre):
    return {
        "nodes_per_core": pre["npc"],
        "n_nodes_total": N_NODES,
        "n_cores": N_CORES,
        "n_graphs": NUM_GRAPHS,
        "chunks_per_tile": pre["K"],
        # HW ACT Lrelu ignores alpha; max(x, 0.2x) is exact and verified
        "lrelu_act": False,
        # feat[dst] is reconstructed on the PE from the one-hot instead of
        # a second indirect gather (indirect DMAs are queue-bound)
        "dst_expand": True,
    }


def kernel(**inputs):
    from concourse.bass_utils import run_bass_kernel_spmd

    src = np.asarray(inputs["src"], np.int32)
    dst = np.asarray(inputs["dst"], np.int32)
    graph_ids = np.asarray(inputs["graph_ids"], np.int32)

    pre = _preprocess(src, dst, graph_ids, N_NODES, N_CORES, NUM_GRAPHS)
    nc = _build_nc(_full_cfg(pre))
    in_maps = _make_in_maps(inputs, pre, N_CORES, NUM_GRAPHS)
    res = run_bass_kernel_spmd(nc, in_maps, list(range(N_CORES)))
    total = np.zeros((NUM_GRAPHS, D), np.float32)
    for r in res.results:
        total += r["gsum"]
    counts = np.bincount(graph_ids, minlength=NUM_GRAPHS).astype(np.float32)
    return (total / np.maximum(counts, 1.0)[:, None]).astype(np.float32)
